# revision 14
# baseline (speedup 1.0000x reference)
"""GumbelSoftmaxMessageDecoder Trainium2 kernel.

LSTM decoder, T=16 steps, B=16384 batch, D=512 hidden, A=65 actions.
Data-parallel over batch: 8 cores x 2048 rows each.

Key algebraic restructurings vs the reference:
  * x @ W_ih.T with x = onehot @ W_emb fuses into onehot @ (W_emb @ W_ih.T):
    contraction drops 512 -> 67, so the recurrent gates matmul contracts over
    579 (=512 h + 67 onehot) instead of 1024.
  * softmax(log_softmax(logits) + gumbel) == softmax(logits + gumbel); the
    logsumexp term only survives in eos_p = exp(logits[:,0]) / sum(exp(logits)).

On-chip layout: hidden state kept transposed [D, B] so the recurrent matmul
needs no per-step transposes; only the tiny [*, 65] logits/action tiles are
transposed (PE) each step to do the softmax along the free axis.
All matmuls run in float32r (full-rate fp32 on the PE).
"""

import os

import numpy as np

import concourse.bacc as bacc
import concourse.bass as bass
import concourse.mybir as mybir
import concourse.tile as tile
from concourse.bass import ds, ts
from concourse.bass_utils import run_bass_kernel_spmd
from concourse.masks import make_identity

AFT = mybir.ActivationFunctionType
f32 = mybir.dt.float32
f32r = mybir.dt.float32r

NCORES = 8
B = 16384
D = 512
T = 16
A = 65            # action space
V = 67            # vocab (action + pad + bos)
BOS = 66
G = 4 * D         # 2048 gate rows
BS = B // NCORES  # 2048 rows per core
NBT = BS // 128   # 16 batch tiles per core
KT = 5            # contraction tiles: 4x128 (h) + 1x128 (onehot, padded 67->128)
BH = BS // 2      # 1024: half-batch tile width for elementwise work
NBC = BS // 512   # 4 matmul N-chunks of 512


def _build(trace_scopes=False):
    nc = bacc.Bacc("TRN2", target_bir_lowering=False, debug=False,
                   num_devices=NCORES)
    enc_d = nc.dram_tensor("enc", [BS, D], f32, kind="ExternalInput").ap()
    gum_d = nc.dram_tensor("gum", [T, BS, A], f32, kind="ExternalInput").ap()
    wg_d = nc.dram_tensor("wg", [KT * 128, G], f32, kind="ExternalInput").ap()
    wa_d = nc.dram_tensor("wa", [D, A], f32, kind="ExternalInput").ap()
    wc_d = nc.dram_tensor("wc", [D, D], f32, kind="ExternalInput").ap()
    wh_d = nc.dram_tensor("wh", [D, D], f32, kind="ExternalInput").ap()
    bg_d = nc.dram_tensor("bg", [128, 16], f32, kind="ExternalInput").ap()
    bg0_d = nc.dram_tensor("bg0", [128, 16], f32, kind="ExternalInput").ap()
    bc_d = nc.dram_tensor("bc", [128, 4], f32, kind="ExternalInput").ap()
    bh_d = nc.dram_tensor("bh", [128, 4], f32, kind="ExternalInput").ap()
    ba_d = nc.dram_tensor("ba", [A, 1], f32, kind="ExternalInput").ap()
    msg_d = nc.dram_tensor("msg", [BS, T + 1, A], f32, kind="ExternalOutput").ap()
    eos_d = nc.dram_tensor("eos", [BS, T + 1], f32, kind="ExternalOutput").ap()

    msg_r = msg_d.rearrange("(bt p) t a -> p bt t a", p=128)
    eos_r = eos_d.rearrange("(bt p) t -> p bt t", p=128)

    with tile.TileContext(nc) as tc:
        consts = tc.alloc_tile_pool(name="consts", bufs=1)
        state = tc.alloc_tile_pool(name="state", bufs=1)

        ident = consts.tile([128, 128], f32)
        make_identity(nc, ident)
        bgs = consts.tile([128, 16], f32)
        nc.sync.dma_start(out=bgs, in_=bg_d)
        bgs0 = consts.tile([128, 16], f32)
        nc.sync.dma_start(out=bgs0, in_=bg0_d)
        bas = consts.tile([A, 1], f32)
        nc.sync.dma_start(out=bas, in_=ba_d)
        eos1 = consts.tile([128, A], f32)
        nc.vector.memset(eos1, 0.0)
        nc.vector.memset(eos1[:, 0:1], 1.0)

        # weights for the recurrent gates matmul, f32r, [128, k, G]
        wg = consts.tile([128, KT, G], f32r)
        wa = consts.tile([128, 4, A], f32r)

        # persistent state
        h = state.tile([128, 4, BS], f32r)     # hidden, [D, B] layout
        c = state.tile([128, 4, BS], f32)      # cell
        hx = state.tile([128, BS], f32r)       # onehot/action transposed [V->128, B]
        eosb = state.tile([128, NBT, T + 1], f32)

        nc.vector.memset(hx.bitcast(f32), 0.0)
        nc.vector.memset(eosb, 1.0)  # col T stays 1 (final eos prob)

        # final message slab: one-hot EOS at t=T
        for bt in range(NBT):
            nc.sync.dma_start(out=msg_r[:, bt, T, :], in_=eos1)

        # ---- init: load weights (convert to f32r), transpose encoded, h0/c0
        with tc.tile_pool(name="wstage", bufs=2) as wstage:
            for k in range(KT):
                stg = wstage.tile([128, G], f32)
                nc.sync.dma_start(out=stg, in_=wg_d[ts(k, 128), :])
                nc.vector.tensor_copy(wg[:, k, :], stg)
            stga = wstage.tile([128, 4, A], f32, tag="stga")
            nc.sync.dma_start(
                out=stga, in_=wa_d.rearrange("(k p) a -> p k a", p=128))
            nc.vector.tensor_copy(wa, stga)

        with tc.tile_pool(name="initp", bufs=2) as initp, \
             tc.tile_pool(name="encTp", bufs=1) as encTp, \
             tc.tile_pool(name="initpsum", bufs=4, space="PSUM") as initpsum:
            encT = encTp.tile([128, 4, BS], f32r)
            enc_r = enc_d.rearrange("(bt p) d -> p bt d", p=128)
            for quad in range(4):   # 4 batch tiles at a time
                ebuf = initp.tile([128, 4, D], f32, tag="ebuf")
                nc.sync.dma_start(out=ebuf, in_=enc_r[:, ts(quad, 4), :])
                for j in range(4):
                    bt = quad * 4 + j
                    for dk in range(4):
                        pt = initpsum.tile([128, 128], f32, tag="tp")
                        nc.tensor.transpose(pt, ebuf[:, j, ts(dk, 128)], ident)
                        nc.scalar.copy(encT[:, dk, ts(bt, 128)], pt)

            # h0 = W_cell @ encT + b_cell ; c0 = W_hid @ encT + b_hid
            for (w_dram, bias_dram, dst, dst_dt) in (
                    (wc_d, bc_d, h, f32r), (wh_d, bh_d, c, f32)):
                wt = initp.tile([128, 4, D], f32r, tag="wt")
                wstg = initp.tile([128, 4, D], f32, tag="wstg")
                nc.sync.dma_start(
                    out=wstg, in_=w_dram.rearrange("(k p) d -> p k d", p=128))
                nc.vector.tensor_copy(wt, wstg)
                bsb = initp.tile([128, 4], f32, tag="bsb")
                nc.sync.dma_start(out=bsb, in_=bias_dram)
                for m in range(4):
                    for bc_ in range(NBC):
                        ps = initpsum.tile([128, 512], f32, tag="mm")
                        for k in range(4):
                            nc.tensor.matmul(
                                ps, wt[:, k, ts(m, 128)],
                                encT[:, k, ts(bc_, 512)],
                                start=(k == 0), stop=(k == 3))
                        nc.scalar.activation(
                            dst[:, m, ts(bc_, 512)], ps, AFT.Identity,
                            bias=bsb[:, m:m + 1])

        # ---- main pools
        gum_pool = tc.alloc_tile_pool(name="gum", bufs=2)
        lt_pool = tc.alloc_tile_pool(name="lt", bufs=2)
        gact = tc.alloc_tile_pool(name="gact", bufs=5)
        tmp_pool = tc.alloc_tile_pool(name="tmp", bufs=2)
        lg_pool = tc.alloc_tile_pool(name="lg", bufs=1)
        sm_pool = tc.alloc_tile_pool(name="sm", bufs=4)
        sc_pool = tc.alloc_tile_pool(name="sc", bufs=8)
        msg_pool = tc.alloc_tile_pool(name="msgp", bufs=2)
        ab_pool = tc.alloc_tile_pool(name="abp", bufs=2)
        pg_pool = tc.alloc_tile_pool(name="pg", bufs=2, space="PSUM")
        pl_pool = tc.alloc_tile_pool(name="pl", bufs=2, space="PSUM")
        pt_pool = tc.alloc_tile_pool(name="ptr", bufs=2, space="PSUM")

        for t in range(T):
            # gumbel input for this step; g2 = ln(-ln u)  (gumbel = -g2)
            gumt = gum_pool.tile([128, NBT, A], f32)
            nc.sync.dma_start(
                out=gumt, in_=gum_d[t].rearrange("(bt p) a -> p bt a", p=128))
            l1 = lt_pool.tile([128, NBT, A], f32, tag="l1")
            g2 = lt_pool.tile([128, NBT, A], f32, tag="g2")
            nc.scalar.activation(l1, gumt, AFT.Ln)
            nc.scalar.activation(g2, l1, AFT.Ln, scale=-1.0)

            # gates + LSTM cell update, per (d-tile, half-batch)
            for d in range(4):
                for half in range(2):
                    chs = ds(half * BH, BH)
                    ga = [None] * 4
                    for kind in range(4):     # i, f, g, o
                        m = kind * 4 + d
                        act = gact.tile([128, BH], f32, tag="gact")
                        fn = AFT.Tanh if kind == 2 else AFT.Sigmoid
                        ps = pg_pool.tile([128, 1024], f32)
                        # t=0: onehot is BOS for every row -> constant,
                        # folded into the bias (bgs0); skip the k=4 matmul.
                        nk = 4 if t == 0 else KT
                        for sub in range(2):
                            bc_ = half * 2 + sub
                            for k in range(nk):
                                rhs = (h[:, k, ts(bc_, 512)] if k < 4
                                       else hx[:, ts(bc_, 512)])
                                nc.tensor.matmul(
                                    ps[:, ts(sub, 512)], wg[:, k, ts(m, 128)],
                                    rhs, start=(k == 0), stop=(k == nk - 1))
                        nc.scalar.activation(act, ps, fn,
                                             bias=(bgs0 if t == 0 else bgs)[:, m:m + 1])
                        ga[kind] = act
                    gi, gf, gg, go = ga
                    # c = f*c + i*g ; h = o * tanh(c)
                    t2 = tmp_pool.tile([128, BH], f32, tag="t2")
                    nc.vector.tensor_mul(t2, gi, gg)
                    nc.vector.tensor_mul(c[:, d, chs], gf, c[:, d, chs])
                    nc.vector.tensor_add(c[:, d, chs], c[:, d, chs], t2)
                    th = tmp_pool.tile([128, BH], f32, tag="th")
                    nc.scalar.activation(th, c[:, d, chs], AFT.Tanh)
                    nc.vector.tensor_mul(h[:, d, chs], go, th)

            # logits = W_act @ h + b_act, [A, BS]
            lg = lg_pool.tile([A, BS], f32)
            for bc_ in range(NBC):
                pl = pl_pool.tile([A, 512], f32)
                for k in range(4):
                    nc.tensor.matmul(pl, wa[:, k, :], h[:, k, ts(bc_, 512)],
                                     start=(k == 0), stop=(k == 3))
                nc.vector.tensor_scalar_add(lg[:, ts(bc_, 512)], pl, bas)

            msg_stage = msg_pool.tile([128, NBT, A], f32)
            abuf = ab_pool.tile([128, NBT, A], f32)
            for bt in range(NBT):
                # transpose logits tile -> [128, A]
                pt = pt_pool.tile([128, 128], f32, tag="tp")
                nc.tensor.transpose(pt[:, 0:A], lg[:, ts(bt, 128)],
                                    ident[0:A, 0:A])
                # eos prob: exp(logits[:,0]) / sum exp(logits)
                e1 = sm_pool.tile([128, A], f32, tag="e1")
                s1 = sc_pool.tile([128, 1], f32, tag="s1")
                nc.scalar.activation(e1, pt[:, 0:A], AFT.Exp, accum_out=s1)
                r1 = sc_pool.tile([128, 1], f32, tag="r1")
                nc.vector.reciprocal(r1, s1)
                nc.vector.tensor_mul(eosb[:, bt, t:t + 1], e1[:, 0:1], r1)
                # action = softmax(logits + gumbel)
                zb = sm_pool.tile([128, A], f32, tag="zb")
                nc.vector.tensor_sub(zb, pt[:, 0:A], g2[:, bt, :])
                e2 = sm_pool.tile([128, A], f32, tag="e2")
                s2 = sc_pool.tile([128, 1], f32, tag="s2")
                nc.scalar.activation(e2, zb, AFT.Exp, accum_out=s2)
                r2 = sc_pool.tile([128, 1], f32, tag="r2")
                nc.vector.reciprocal(r2, s2)
                nc.vector.tensor_scalar_mul(msg_stage[:, bt, :], e2, r2)
                nc.vector.tensor_scalar_mul(abuf[:, bt, :], e2, r2)
                # transpose action back into hx for the next step
                pa = pt_pool.tile([128, 128], f32, tag="tp")
                nc.tensor.transpose(pa[0:A, :], abuf[:, bt, :], ident)
                nc.vector.tensor_copy(hx[0:A, ts(bt, 128)], pa[0:A, :])

            nc.sync.dma_start(out=msg_r[:, :, t, :], in_=msg_stage)

        nc.sync.dma_start(out=eos_r, in_=eosb)

        for p in (pt_pool, pl_pool, pg_pool, ab_pool, msg_pool, sc_pool,
                  sm_pool, lg_pool, tmp_pool, gact, lt_pool, gum_pool,
                  state, consts):
            p.release()

    nc.compile()
    return nc


_CACHE = {}


def _get_nc():
    if "nc" not in _CACHE:
        _CACHE["nc"] = _build()
    return _CACHE["nc"]


def kernel(encoded, gumbel_u, W_emb, W_cell, b_cell, W_hid, b_hid,
           W_ih, W_hh, b_ih, b_hh, W_act, b_act):
    encoded = np.ascontiguousarray(np.asarray(encoded, np.float32))
    gumbel_u = np.ascontiguousarray(np.asarray(gumbel_u, np.float32))

    # host-side weight layout prep (tiny)
    M = np.asarray(W_emb, np.float64) @ np.asarray(W_ih, np.float64).T  # [V, G]
    wg_full = np.zeros((KT * 128, G), np.float32)
    wg_full[:D] = np.asarray(W_hh, np.float32).T
    wg_full[D:D + V] = M.astype(np.float32)
    wa_h = np.ascontiguousarray(np.asarray(W_act, np.float32).T)       # [D, A]
    wc_h = np.ascontiguousarray(np.asarray(W_cell, np.float32).T)      # [D, D]
    wh_h = np.ascontiguousarray(np.asarray(W_hid, np.float32).T)
    bgv = np.asarray(b_ih, np.float64) + np.asarray(b_hh, np.float64)
    bg_h = np.ascontiguousarray(bgv.astype(np.float32).reshape(16, 128).T)
    bg0_h = np.ascontiguousarray(
        (bgv + M[BOS]).astype(np.float32).reshape(16, 128).T)
    bc_h = np.ascontiguousarray(np.asarray(b_cell, np.float32).reshape(4, 128).T)
    bh_h = np.ascontiguousarray(np.asarray(b_hid, np.float32).reshape(4, 128).T)
    ba_h = np.ascontiguousarray(np.asarray(b_act, np.float32).reshape(A, 1))

    shared = {"wg": wg_full, "wa": wa_h, "wc": wc_h, "wh": wh_h,
              "bg": bg_h, "bg0": bg0_h, "bc": bc_h, "bh": bh_h, "ba": ba_h}
    in_maps = []
    for i in range(NCORES):
        sl = slice(i * BS, (i + 1) * BS)
        in_maps.append({"enc": encoded[sl],
                        "gum": np.ascontiguousarray(gumbel_u[:, sl]),
                        **shared})

    nc = _get_nc()
    trace = bool(os.environ.get("KERNEL_TRACE"))
    try:
        res = run_bass_kernel_spmd(nc, in_maps, core_ids=list(range(NCORES)),
                                   trace=trace)
    except (ImportError, ModuleNotFoundError):
        # no NTFF profiling hook in this environment; run untraced
        trace = False
        res = run_bass_kernel_spmd(nc, in_maps, core_ids=list(range(NCORES)))
    if trace and res.exec_time_ns is not None:
        _CACHE["last_results"] = res
        print("HW exec time:", res.exec_time_ns, "ns")
        if res.instructions_and_trace is not None:
            print("trace path:", res.instructions_and_trace[1])

    message = np.concatenate([r["msg"] for r in res.results], axis=0)
    eos_probs = np.concatenate([r["eos"] for r in res.results], axis=0)
    return message, eos_probs
